# revision 8
# baseline (speedup 1.0000x reference)
"""Multi-head self-attention Trainium2 kernel (B=2, S=2048, D=1024, H=32, d=32).

Sharding: 8 cores = (batch b in {0,1}) x (query quarter qc in {0..3}).
Each core holds x[b].T fully (keys) and computes attention + output
projection for its 512 queries. Per-core inputs are column-rolled so the
core's queries are always columns 0..511 (softmax is key-order invariant,
so rolling keys is safe). Host concatenates the per-core outputs.

Per-core pipeline (bf16 operands, fp32 PSUM accumulation):
  scores via the folded matrix M = wq @ wk.T: scoresT = x_k^T (M^T x_q),
  so only the query side is projected (mq = blockdiag(M)^T @ xq) and the
  key-side lhsT is raw XT — no K projection at all. Scores in [keys, q]
  orientation via PE row-tiling (tile_position (32j, 0)); exp on ACT with
  fused 1/sqrt(d) scale (no max subtraction; |s| <= ~13 for randn inputs);
  v = blockdiag(wv) projection with a ones column appended per head so
  attn@v accumulates out[(e|sum), q] per head at (bank j//2, col strip
  64*(j%2)); per-head softmax denominators land in the strip+32 row.
  PSUM accumulator banks are opened by a zero matmul (start=True clears
  has_written bank-wide, the zero write re-arms accumulation), then all
  attn@v matmuls accumulate with start=False.
  Tail per head group (deferred one group): evacuate po, bulk reciprocal,
  broadcast 1/den onto each 64-row strip via a 1-partition ones matmul,
  multiply the 32 e-rows into strip-layout CT chunks (dead CT rows are
  zeroed once; wo is host-reordered and zero-padded to match). Output
  projection contracts the 16 chunks against wop accumulating in PSUM.
"""
import numpy as np
import ml_dtypes

import concourse.bacc as bacc
import concourse.mybir as mybir
import concourse.tile as tile
from concourse import bass_utils

f32 = mybir.dt.float32
bf16 = mybir.dt.bfloat16
AF = mybir.ActivationFunctionType

B, S, D, H, dh = 2, 2048, 1024, 32, 32
NCORES = 8
QCH = S // (NCORES // B)      # 512 queries per core
NHG = D // 128                # 8 four-head groups
NKC = S // 128                # 16 key chunks
NQS = QCH // 128              # 4 query sub-chunks
SCALE = 1.0 / float(np.sqrt(dh))


def build_module(loop_iters: int = 0, stage: int = 6):
    nc = bacc.Bacc("TRN2", target_bir_lowering=False, debug=False)
    xt_d = nc.dram_tensor("xt", [D, S], f32, kind="ExternalInput")
    mbd_d = nc.dram_tensor("mbd", [128, 128], bf16, kind="ExternalInput")
    wvbd_d = nc.dram_tensor("wvbd", [128, 128], bf16, kind="ExternalInput")
    wop_d = nc.dram_tensor("wop", [16 * 128, D], bf16, kind="ExternalInput")
    out_d = nc.dram_tensor("out", [QCH, D], f32, kind="ExternalOutput")

    with tile.TileContext(nc) as tc:
        with (
            tc.tile_pool(name="pers", bufs=1) as pers,
            tc.tile_pool(name="sbm", bufs=2) as sbm,
            tc.tile_pool(name="sbe", bufs=6) as sbe,
            tc.tile_pool(name="sbv", bufs=6) as sbv,
            tc.tile_pool(name="sbx", bufs=2) as sbx,
            tc.tile_pool(name="psS", bufs=2, space="PSUM") as psS,
            tc.tile_pool(name="psO", bufs=1, space="PSUM") as psO,
            tc.tile_pool(name="psA", bufs=2, space="PSUM") as psA,
        ):
            def body(_iv=None):
                XT = []
                for t in range(NHG):
                    xtt = pers.tile([128, S], bf16, name=f"XT{t}", tag=f"XT{t}")
                    nc.gpsimd.dma_start(xtt[:, :], xt_d[128 * t:128 * (t + 1), :])
                    XT.append(xtt)
                WOP = []
                for t in range(16):
                    wot = pers.tile([128, D], bf16, name=f"WOP{t}",
                                    tag=f"WOP{t}")
                    nc.sync.dma_start(wot[:, :], wop_d[128 * t:128 * (t + 1), :])
                    WOP.append(wot)
                mbd = pers.tile([128, 128], bf16, name="mbd", tag="mbd")
                nc.sync.dma_start(mbd[:, :], mbd_d[:, :])
                wvbd = pers.tile([128, 128], bf16, name="wvbd", tag="wvbd")
                nc.sync.dma_start(wvbd[:, :], wvbd_d[:, :])
                zrow = pers.tile([1, 640], bf16, name="zrow", tag="zrow")
                nc.vector.memset(zrow[:, :], 0.0)
                onesf = pers.tile([128, 64], f32, name="onesf", tag="onesf")
                nc.vector.memset(onesf[:, :], 1.0)

                if stage <= 1:
                    sink = pers.tile([128, 128], bf16, name="sink", tag="sink")
                    for t in range(NHG):
                        nc.vector.tensor_copy(sink[:, :], XT[t][:, 0:128])
                    for t in range(16):
                        nc.vector.tensor_copy(sink[:, :], WOP[t][:, 0:128])
                    nc.vector.tensor_copy(sink[:, :], mbd[:, :])
                    nc.vector.tensor_copy(sink[:, :], wvbd[:, :])
                    sinkf = pers.tile([128, 128], f32, name="sinkf",
                                      tag="sinkf")
                    nc.vector.tensor_copy(sinkf[:, :], sink[:, :])
                    nc.sync.dma_start(out_d[0:128, 0:128], sinkf[:, :])
                    return

                CT = []
                OUTSB = []
                if stage >= 6:
                    for c in range(16):
                        ctt = pers.tile([128, QCH], bf16, name=f"CT{c}",
                                        tag=f"CT{c}")
                        # zero once: dead rows (32:64, 96:128) must read 0
                        # for the projection (wop rows there are zero too,
                        # but stale NaNs would poison 0*NaN)
                        nc.vector.memset(ctt[:, :], 0.0)
                        CT.append(ctt)
                    for qs in range(NQS):
                        ot = pers.tile([128, D], f32, name=f"OUTSB{qs}",
                                       tag=f"OUTSB{qs}")
                        OUTSB.append(ot)

                pending_tails = []
                for hg in range(NHG):
                    # ---- mq = blockdiag(M)^T @ xq (queries are cols 0..511)
                    pmq = psA.tile([128, 512], f32, name=f"pmq{hg}", tag="aux")
                    nc.tensor.matmul(pmq[:, :], mbd[:, :], XT[hg][:, 0:QCH],
                                     start=True, stop=True)
                    mqt = sbm.tile([128, QCH], bf16, name=f"mqt{hg}", tag="mq")
                    nc.vector.tensor_copy(mqt[:, :], pmq[:, :])

                    # ---- V tiles: [128 keys, 4 kc x (4 heads x 33)],
                    # ones column at 33j+32 per head
                    vts = []
                    for kq in range(4):
                        pv = psA.tile([128, 512], f32, name=f"pv{hg}_{kq}",
                                      tag="aux")
                        for u in range(4):
                            kc = 4 * kq + u
                            nc.tensor.matmul(
                                pv[:, 128 * u:128 * (u + 1)],
                                XT[hg][:, 128 * kc:128 * (kc + 1)],
                                wvbd[:, :],
                                start=(u == 0), stop=(u == 3),
                                skip_group_check=True)
                        vt = sbv.tile([128, 528], bf16, name=f"vt{hg}_{kq}",
                                      tag="v")
                        nc.vector.tensor_copy(
                            vt[:, :].rearrange("p (c h e) -> p c h e",
                                               c=4, h=4)[:, :, :, 0:32],
                            pv[:, :].rearrange("p (c h e) -> p c h e",
                                               c=4, h=4))
                        nc.vector.memset(
                            vt[:, :].rearrange("p (c h e) -> p c h e",
                                               c=4, h=4)[:, :, :, 32:33], 1.0)
                        vts.append(vt)
                    if stage <= 2:
                        continue

                    # ---- attn@v accumulator: open both banks via zero
                    # matmuls (start=True clears has_written bank-wide; the
                    # zero write re-arms it so start=False accumulates)
                    po = psO.tile([128, 1024], f32, name=f"po{hg}", tag="o")
                    if stage >= 5:
                        for bank in range(2):
                            nc.tensor.matmul(po[:, 512 * bank:512 * (bank + 1)],
                                             zrow[:, 0:128], zrow[:, 128:640],
                                             start=True, stop=True,
                                             skip_group_check=True)

                    def attnv(kc, ets_kc):
                        vt = vts[kc // 4]
                        base = 132 * (kc % 4)
                        for j in (0, 2, 1, 3):
                            nc.tensor.matmul(
                                po[:, 512 * (j // 2):512 * (j // 2) + 512][
                                    64 * (j % 2):64 * (j % 2) + 33, :],
                                vt[:, base + 33 * j:base + 33 * (j + 1)],
                                ets_kc[j // 2][:, 512 * (j % 2):
                                               512 * (j % 2) + 512],
                                start=False, stop=(kc == NKC - 1),
                                skip_group_check=True)

                    prev = None
                    for kc in range(NKC):
                        sss = []
                        for pr in range(2):
                            ss = psS.tile([128, 1024], f32,
                                          name=f"ss{hg}_{kc}_{pr}", tag="s")
                            for jj in range(2):
                                j = 2 * pr + jj
                                nc.tensor.matmul(
                                    ss[:, 512 * jj:512 * (jj + 1)],
                                    XT[hg][32 * j:32 * (j + 1),
                                           128 * kc:128 * (kc + 1)],
                                    mqt[32 * j:32 * (j + 1), :],
                                    start=True, stop=True,
                                    tile_position=(32 * j, 0))
                            sss.append(ss)
                        ets = []
                        for pr in range(2):
                            et = sbe.tile([128, 1024], bf16,
                                          name=f"et{hg}_{kc}_{pr}", tag="e")
                            if stage <= 3:
                                nc.vector.tensor_copy(et[:, :], sss[pr][:, :])
                            else:
                                nc.scalar.activation(et[:, :], sss[pr][:, :],
                                                     AF.Exp, scale=SCALE)
                            ets.append(et)
                        if stage >= 5 and prev is not None:
                            attnv(prev[0], prev[1])
                        prev = (kc, ets)
                    if stage >= 5:
                        attnv(prev[0], prev[1])
                    if stage <= 5:
                        continue

                    # ---- evacuate po once (releases the accumulator banks
                    # for the next head group); normalize is DEFERRED one
                    # head group so its sem chains never stall the next
                    # group's score/attn stream
                    pof = sbx.tile([128, 1024], f32, name=f"pof{hg}",
                                   tag="pof", bufs=3)
                    nc.vector.tensor_copy(pof[:, :], po[:, :])

                    def make_tail(hg, pof):
                        def tail():
                            rof = sbx.tile([128, 1024], f32, name=f"rof{hg}",
                                           tag="rof")
                            nc.vector.reciprocal_approx_fast(rof[:, :],
                                                             pof[:, :])
                            for bank in range(2):
                                c = 2 * hg + bank
                                pb = psA.tile([128, 512], f32,
                                              name=f"pb{hg}_{bank}",
                                              tag="aux")
                                for sj in range(2):
                                    strip = 64 * sj
                                    nc.tensor.matmul(
                                        pb[strip:strip + 64, :],
                                        onesf[strip + 32:strip + 33, :],
                                        rof[strip + 32:strip + 33,
                                            512 * bank:512 * bank + 512],
                                        start=True, stop=True,
                                        tile_position=(strip + 32, strip))
                                for sj in range(2):
                                    strip = 64 * sj
                                    nc.vector.tensor_mul(
                                        CT[c][strip:strip + 32, :],
                                        pof[strip:strip + 32,
                                            512 * bank:512 * bank + 512],
                                        pb[strip:strip + 32, :])
                        return tail

                    pending_tails.append(make_tail(hg, pof))
                    if len(pending_tails) > 1:
                        pending_tails.pop(0)()

                if stage <= 5:
                    return
                for t_ in pending_tails:
                    t_()

                # ---- output projection: contract 16 chunks in PSUM
                for qs in range(NQS):
                    for og in range(2):
                        pe_ = psA.tile([128, 512], f32, name=f"pe{qs}_{og}",
                                       tag="aux")
                        for c in range(16):
                            nc.tensor.matmul(
                                pe_[:, :],
                                CT[c][:, 128 * qs:128 * (qs + 1)],
                                WOP[c][:, 512 * og:512 * (og + 1)],
                                start=(c == 0), stop=(c == 15))
                        nc.vector.tensor_copy(
                            OUTSB[qs][:, 512 * og:512 * (og + 1)], pe_[:, :])
                for qs in range(NQS):
                    nc.sync.dma_start(out_d[128 * qs:128 * (qs + 1), :],
                                      OUTSB[qs][:, :])

            if loop_iters > 0:
                with tc.For_i(0, loop_iters, 1):
                    body()
            else:
                body()

    nc.compile()
    return nc


def _prep_inputs(x, wq, bq, wk, bk, wv, bv, wo, bo):
    x = np.asarray(x, dtype=np.float32)
    wq = np.asarray(wq, dtype=np.float32)
    wk = np.asarray(wk, dtype=np.float32)
    wv = np.asarray(wv, dtype=np.float32)
    wo = np.asarray(wo, dtype=np.float32)
    for name, b_ in (("bq", bq), ("bk", bk), ("bv", bv)):
        if np.any(np.asarray(b_) != 0):
            raise NotImplementedError(f"nonzero {name} not supported")

    def blockdiag(w):
        o = np.zeros((128, 128), np.float32)
        for i in range(4):
            o[32 * i:32 * (i + 1), 32 * i:32 * (i + 1)] = w
        return o

    # wo rows reordered+zero-padded to match the strip-layout CT chunks:
    # head h = 4*hg + jm -> chunk c = 2*hg + jm//2, strip 64*(jm%2)
    wop = np.zeros((16 * 128, D), np.float32)
    for h in range(H):
        hg, jm = h // 4, h % 4
        c = 2 * hg + (jm // 2)
        strip = 64 * (jm % 2)
        wop[128 * c + strip:128 * c + strip + 32, :] = wo[32 * h:32 * (h + 1), :]

    bfl = ml_dtypes.bfloat16
    m = wq @ wk.T
    shared = {
        "mbd": blockdiag(m).astype(bfl),
        "wvbd": blockdiag(wv).astype(bfl),
        "wop": wop.astype(bfl),
    }
    xts = [np.ascontiguousarray(x[b].T) for b in range(B)]
    in_maps = []
    for c in range(NCORES):
        b, qc = c // (NCORES // B), c % (NCORES // B)
        mm = dict(shared)
        # roll keys so this core's queries are columns 0..511
        mm["xt"] = np.ascontiguousarray(np.roll(xts[b], -QCH * qc, axis=1))
        in_maps.append(mm)
    return in_maps


_NC_CACHE = {}


def kernel(x, wq, bq, wk, bk, wv, bv, wo, bo):
    in_maps = _prep_inputs(x, wq, bq, wk, bk, wv, bv, wo, bo)
    if "nc" not in _NC_CACHE:
        _NC_CACHE["nc"] = build_module()
    nc = _NC_CACHE["nc"]
    res = bass_utils.run_bass_kernel_spmd(nc, in_maps,
                                          core_ids=list(range(NCORES)))
    out = np.empty((B, S, D), np.float32)
    for c in range(NCORES):
        b, qc = c // (NCORES // B), c % (NCORES // B)
        out[b, QCH * qc:QCH * (qc + 1), :] = res.results[c]["out"]
    out += np.asarray(bo, dtype=np.float32)[None, None, :]
    return out
